# revision 10
# baseline (speedup 1.0000x reference)
"""Trainium2 Bass kernel for nn_LoopModel2: out = x + sum(range(y)).

The loop `for i in range(y): x = x + i` collapses to a single elementwise
add of the constant y*(y-1)/2 (2016.0 for y=64). The kernel is a pure
HBM-streaming problem: DMA tiles of x into SBUF, add the constant, DMA
back out. x (8192, 8192) f32 is sharded row-wise across 8 NeuronCores;
no communication is needed.
"""

import os

import numpy as np

import concourse.bacc as bacc
import concourse.mybir as mybir
from concourse.tile import TileContext
from concourse.bass_utils import run_bass_kernel_spmd

N_CORES = 8
ROWS, COLS = 8192, 8192
SHARD_ROWS = ROWS // N_CORES  # 1024 rows per core

# Tiling of one core's 32 MiB shard: NT tiles of [P, F] f32.
P = 128
F = int(os.environ.get("KF", 8192))
NT = (SHARD_ROWS * COLS) // (P * F)
BUFS = int(os.environ.get("KBUFS", 5))
# Loads ride the SP HWDGE ring (nc.sync); stores the ACT ring
# (nc.scalar) so both queue rows feed the 16 SDMA engines.
STORE_ENG = os.environ.get("KSTORE", "scalar")
# KALT=1: alternate load/store rings by tile parity so both HWDGE rings
# carry work from t=0 (kills the single-ring ramp phase).
ALT = bool(int(os.environ.get("KALT", "0")))

# Filled in by the last traced run (test.py reads this).
LAST_EXEC_NS = None
LAST_RESULTS = None

_cache = {}


def _build(const: float):
    # Bacc (not raw Bass): its finalize() runs generate_event_semaphores,
    # which splits multi-semaphore waits off DMA/compute instructions —
    # walrus codegen rejects >1 inline sync wait per instruction.
    nc = bacc.Bacc()
    x_in = nc.dram_tensor("x", [NT, P, F], mybir.dt.float32, kind="ExternalInput")
    out = nc.dram_tensor("out", [NT, P, F], mybir.dt.float32, kind="ExternalOutput")

    with TileContext(nc) as tc:
        with tc.tile_pool(name="io", bufs=BUFS) as pool:
            for i in range(NT):
                t = pool.tile([P, F], mybir.dt.float32)
                if ALT:
                    load_eng = nc.sync if i % 2 == 0 else nc.scalar
                    store_eng = nc.scalar if i % 2 == 0 else nc.sync
                else:
                    load_eng = nc.sync
                    store_eng = getattr(nc, STORE_ENG)
                load_eng.dma_start(out=t[:], in_=x_in[i])
                nc.vector.tensor_scalar_add(t[:], t[:], const)
                store_eng.dma_start(out=out[i], in_=t[:])
    nc.finalize()
    return nc


def kernel(x, y) -> np.ndarray:
    global LAST_EXEC_NS, LAST_RESULTS
    y = int(y)
    const = float(y * (y - 1) // 2)

    if const not in _cache:
        _cache[const] = _build(const)
    nc = _cache[const]

    x_np = np.asarray(x, dtype=np.float32)
    in_maps = [
        {"x": x_np[c * SHARD_ROWS:(c + 1) * SHARD_ROWS].reshape(NT, P, F)}
        for c in range(N_CORES)
    ]
    trace = bool(os.environ.get("KERNEL_TRACE"))
    res = run_bass_kernel_spmd(nc, in_maps, list(range(N_CORES)), trace=trace)
    LAST_EXEC_NS = res.exec_time_ns
    LAST_RESULTS = res

    out = np.empty((ROWS, COLS), dtype=np.float32)
    for c in range(N_CORES):
        out[c * SHARD_ROWS:(c + 1) * SHARD_ROWS] = (
            res.results[c]["out"].reshape(SHARD_ROWS, COLS)
        )
    return out


# revision 15
# speedup vs baseline: 1.1695x; 1.1695x over previous
"""Trainium2 Bass kernel for nn_LoopModel2: out = x + sum(range(y)).

The loop `for i in range(y): x = x + i` collapses to a single elementwise
add of the constant y*(y-1)/2 (2016.0 for y=64). The kernel is a pure
HBM-streaming problem: DMA tiles of x into SBUF, add the constant, DMA
back out. x (8192, 8192) f32 is sharded row-wise across 8 NeuronCores;
no communication is needed.
"""

import os

import numpy as np

import concourse.bacc as bacc
import concourse.mybir as mybir
from concourse.tile import TileContext
from concourse.bass_utils import run_bass_kernel_spmd

N_CORES = 8
ROWS, COLS = 8192, 8192
SHARD_ROWS = ROWS // N_CORES  # 1024 rows per core

# Tiling of one core's 32 MiB shard: NT tiles of [P, F] f32.
P = 128
F = int(os.environ.get("KF", 8192))
NT = (SHARD_ROWS * COLS) // (P * F)
BUFS = int(os.environ.get("KBUFS", 5))
# Loads ride the SP HWDGE ring (nc.sync); stores the ACT ring
# (nc.scalar) so both queue rows feed the 16 SDMA engines.
STORE_ENG = os.environ.get("KSTORE", "scalar")
# KALT=1: alternate load/store rings by tile parity so both HWDGE rings
# carry work from t=0 (kills the single-ring ramp phase).
ALT = bool(int(os.environ.get("KALT", "0")))
# KMODE=raw: hand-rolled semaphore pipeline (no Tile barriers).
MODE = os.environ.get("KMODE", "tile")

# Filled in by the last traced run (test.py reads this).
LAST_EXEC_NS = None
LAST_RESULTS = None

_cache = {}


def _build(const: float):
    # Bacc (not raw Bass): its finalize() runs generate_event_semaphores,
    # which splits multi-semaphore waits off DMA/compute instructions —
    # walrus codegen rejects >1 inline sync wait per instruction.
    nc = bacc.Bacc()
    x_in = nc.dram_tensor("x", [NT, P, F], mybir.dt.float32, kind="ExternalInput")
    out = nc.dram_tensor("out", [NT, P, F], mybir.dt.float32, kind="ExternalOutput")

    with TileContext(nc) as tc:
        with tc.tile_pool(name="io", bufs=BUFS) as pool:
            for i in range(NT):
                t = pool.tile([P, F], mybir.dt.float32)
                if ALT:
                    load_eng = nc.sync if i % 2 == 0 else nc.scalar
                    store_eng = nc.scalar if i % 2 == 0 else nc.sync
                else:
                    load_eng = nc.sync
                    store_eng = getattr(nc, STORE_ENG)
                load_eng.dma_start(out=t[:], in_=x_in[i])
                nc.vector.tensor_scalar_add(t[:], t[:], const)
                store_eng.dma_start(out=out[i], in_=t[:])
    nc.finalize()
    return nc


def _build_raw(const: float):
    """Raw-Bass three-engine pipeline: sync loads -> vector adds -> scalar
    stores, explicit semaphores, no Tile drain/butterfly barriers."""
    from contextlib import ExitStack

    nc = bacc.Bacc()
    x_in = nc.dram_tensor("x", [NT, P, F], mybir.dt.float32, kind="ExternalInput")
    out = nc.dram_tensor("out", [NT, P, F], mybir.dt.float32, kind="ExternalOutput")

    with ExitStack() as ctx:
        tiles = [
            ctx.enter_context(nc.sbuf_tensor(f"t{j}", [P, F], mybir.dt.float32))
            for j in range(BUFS)
        ]
        # Per-slot load/store semaphores: DMA completions are NOT ordered
        # across transfers, so a cumulative counter would let add(i) fire
        # when load(i+1) lands before load(i).
        load_sems = [
            ctx.enter_context(nc.semaphore(f"load_sem{j}")) for j in range(BUFS)
        ]
        store_sems = [
            ctx.enter_context(nc.semaphore(f"store_sem{j}")) for j in range(BUFS)
        ]
        add_sem = ctx.enter_context(nc.semaphore("add_sem"))

        # Semaphores are NOT zeroed on allocation; clear them before any
        # engine gates on them, then barrier so no engine races ahead.
        for s in load_sems + store_sems:
            nc.sync.sem_clear(s)
        nc.sync.sem_clear(add_sem)
        nc.all_engine_barrier()

        block = ctx.enter_context(nc.Block())

        @block.sync
        def _(sync):
            for i in range(NT):
                s = i % BUFS
                if i >= BUFS:
                    # Slot reuse: the store that last read this slot is done.
                    sync.wait_ge(store_sems[s], 16 * (i // BUFS))
                sync.dma_start(out=tiles[s][:], in_=x_in[i]).then_inc(
                    load_sems[s], 16
                )

        @block.vector
        def _(vector):
            for i in range(NT):
                s = i % BUFS
                vector.wait_ge(load_sems[s], 16 * (i // BUFS + 1))
                vector.tensor_scalar_add(tiles[s][:], tiles[s][:], const).then_inc(
                    add_sem, 1
                )

        @block.scalar
        def _(scalar):
            for i in range(NT):
                s = i % BUFS
                scalar.wait_ge(add_sem, i + 1)
                scalar.dma_start(out=out[i], in_=tiles[s][:]).then_inc(
                    store_sems[s], 16
                )
            # Make sure the last stores' bytes are committed before the
            # NEFF is considered done.
            for s in range(BUFS):
                n_s = len([i for i in range(NT) if i % BUFS == s])
                scalar.wait_ge(store_sems[s], 16 * n_s)

    nc.finalize()
    return nc


def kernel(x, y) -> np.ndarray:
    global LAST_EXEC_NS, LAST_RESULTS
    y = int(y)
    const = float(y * (y - 1) // 2)

    key = (const, MODE)
    if key not in _cache:
        _cache[key] = _build_raw(const) if MODE == "raw" else _build(const)
    nc = _cache[key]

    x_np = np.asarray(x, dtype=np.float32)
    in_maps = [
        {"x": x_np[c * SHARD_ROWS:(c + 1) * SHARD_ROWS].reshape(NT, P, F)}
        for c in range(N_CORES)
    ]
    trace = bool(os.environ.get("KERNEL_TRACE"))
    res = run_bass_kernel_spmd(nc, in_maps, list(range(N_CORES)), trace=trace)
    LAST_EXEC_NS = res.exec_time_ns
    LAST_RESULTS = res

    out = np.empty((ROWS, COLS), dtype=np.float32)
    for c in range(N_CORES):
        out[c * SHARD_ROWS:(c + 1) * SHARD_ROWS] = (
            res.results[c]["out"].reshape(SHARD_ROWS, COLS)
        )
    return out
